# revision 7
# baseline (speedup 1.0000x reference)
"""GNN message passing (segment-sum + segment-product) on 8 TRN2 NeuronCores.

Strategy (node sharding, no collectives):
- dst nodes are grouped into 256-node windows; windows are LPT-packed onto
  the 8 devices to balance edge counts (the host scatters outputs back by a
  window map). Each device only computes rows for windows it owns.
- The node-feature table is replicated: fp16 rows [64 x_sum feats |
  64 ln(x_prod + 1e-38) feats] (ln precomputed on host, so the device never
  runs Ln; the gather is descriptor-rate-bound, so fp16's smaller rows are
  free and halve vector-engine work downstream).
- Edges are fetched with GPSIMD dma_gather (256B rows), chunks alternating
  across 2 SWDGE queues (the per-queue descriptor processor is the serial
  resource; two queues run in parallel). dma_gather indices are int16, so
  the table is addressed through two views (rows < 32768 and the rest);
  each window's edges are ordered low-rows-first.
- Padding slots are skipped: their idx is -1 (trailing negatives are not
  processed by the gather) and the true per-chunk count is loaded into a
  Pool register for num_idxs_reg, so each device gathers only real edges.
  Padding dstrel is -1, which zeroes the one-hot column, so whatever is in
  the skipped SBUF slots contributes nothing (the message arena is memset
  once at start so it is always finite).
- Segment-sum is a one-hot matmul: onehot[e, n] = (dstrel[e] == n) built by
  the vector engine in fp16, then PE computes msg.T @ onehot accumulated in
  PSUM per window. The product path accumulates ln-sums; Act applies Exp on
  evacuation.
- Device output is feature-major [128, windows*256]; rows 0:64 sums, rows
  64:128 products. The host scatters window columns back and transposes.
"""

import time

import numpy as np

import concourse.bacc as bacc
import concourse.mybir as mybir
import concourse.tile as tile

F = 128          # table row width: 64 sum feats | 64 ln(prod) feats
P = 128
NW = 256         # dst nodes per window
SPLIT = 32768    # int16 index limit for dma_gather
LN_BIAS = 1e-38
N_DEVICES = 8
G_GATHER = 7     # tiles per dma_gather
G_OH = 7         # tiles per one-hot batch
NQ = 2           # SWDGE queues

_MAX_WAITS = 1   # this walrus build allows one sync wait per instruction


def _split_multi_waits(nc):
    """Split instructions carrying more sem waits than walrus accepts."""
    for fn in nc.m.functions:
        for bb in fn.blocks:
            insts = list(bb.instructions)
            new_insts = []
            changed = False
            for inst in insts:
                si = inst.sync_info
                if si is not None and len(si.on_wait) > _MAX_WAITS:
                    waits = list(si.on_wait)
                    k = 0
                    while len(waits) > _MAX_WAITS:
                        chunk, waits = waits[:_MAX_WAITS], waits[_MAX_WAITS:]
                        helper = mybir.InstDrain(
                            name=f"{inst.name}_ws{k}", engine=inst.engine)
                        helper.sync_info = mybir.SyncInfo(
                            on_wait=chunk, on_update=[])
                        new_insts.append(helper)
                        k += 1
                    inst.sync_info = mybir.SyncInfo(
                        on_wait=waits, on_update=list(si.on_update))
                    changed = True
                new_insts.append(inst)
            if changed:
                bb.instructions = new_insts


def make_chunks(T_los, T_his, g_gather=G_GATHER):
    """Gather chunk list shared by builder and host prep: (t0, g, hi)."""
    run_list = []
    pos = 0
    for w in range(len(T_los)):
        run_list.append((pos, T_los[w], False))
        run_list.append((pos + T_los[w], T_his[w], True))
        pos += T_los[w] + T_his[w]
    chunks = []
    for run_start, run_len, hi in run_list:
        if run_len == 0:
            continue
        n_chunks = -(-run_len // g_gather)
        base_sz, extra = divmod(run_len, n_chunks)
        t0 = run_start
        for j in range(n_chunks):
            g = base_sz + (1 if j < extra else 0)
            chunks.append((t0, g, hi))
            t0 += g
    return chunks


def _build_kernel(R, T, n_windows, T_los, T_his, reps=1, nq=NQ,
                  g_gather=G_GATHER, g_oh=G_OH, mbufs=6, obufs=3, pbufs=8,
                  skip_pad=True):
    T_los = tuple(T_los)
    T_his = tuple(T_his)
    assert T == sum(T_los) + sum(T_his)
    tile_win = []
    tile_tw = []
    for w in range(n_windows):
        for tw in range(T_los[w] + T_his[w]):
            tile_win.append(w)
            tile_tw.append(tw)
    chunks = make_chunks(T_los, T_his, g_gather)

    nc = bacc.Bacc("TRN2", target_bir_lowering=False, debug=False,
                   num_swdge_queues=nq)
    f32 = mybir.dt.float32
    f16 = mybir.dt.float16
    i16 = mybir.dt.int16

    table = nc.dram_tensor("table", [R, F], f16, kind="ExternalInput").ap()
    idx16 = nc.dram_tensor("idx16", [P, T * 8], i16,
                           kind="ExternalInput").ap()
    dstrel = nc.dram_tensor("dstrel", [P, T], f16, kind="ExternalInput").ap()
    iota = nc.dram_tensor("iota", [P, g_oh * NW], f16,
                          kind="ExternalInput").ap()
    counts = nc.dram_tensor("counts", [1, len(chunks)], mybir.dt.int32,
                            kind="ExternalInput").ap()
    out = nc.dram_tensor("out", [P, n_windows * NW], f32,
                         kind="ExternalOutput").ap()

    with tile.TileContext(nc) as tc:
        with (
            tc.tile_pool(name="const", bufs=1) as cpool,
            tc.tile_pool(name="msg", bufs=mbufs) as mpool,
            tc.tile_pool(name="oh", bufs=obufs) as opool,
            tc.tile_pool(name="outb", bufs=1) as outpool,
            tc.tile_pool(name="psum", bufs=pbufs, space="PSUM") as ppool,
        ):
            idx_sb = cpool.tile([P, T * 8], i16, tag="idx")
            counts_sb = cpool.tile([1, len(chunks)], mybir.dt.int32,
                                   tag="counts")
            dstrel_sb = cpool.tile([P, T], f16, tag="dstrel")
            iota_sb = cpool.tile([P, g_oh * NW], f16, tag="iota")
            nc.sync.dma_start(out=idx_sb[:], in_=idx16[:])
            nc.sync.dma_start(out=counts_sb[:], in_=counts[:])
            nc.sync.dma_start(out=dstrel_sb[:], in_=dstrel[:])
            nc.sync.dma_start(out=iota_sb[:], in_=iota[:])
            cnt_reg = nc.gpsimd.alloc_register("gather_cnt")
            outbuf = outpool.tile([P, n_windows * NW], f32, tag="outbuf")

            chunk_of_tile = {}
            for ci, (t0, g, hi) in enumerate(chunks):
                for j in range(g):
                    chunk_of_tile[t0 + j] = (ci, j)
            queue_of_chunk = []
            qload = [0] * nq
            for (t0, g, hi) in chunks:
                q = min(range(nq), key=lambda i: qload[i])
                queue_of_chunk.append(q)
                qload[q] += g

            if skip_pad:
                # make the message arena finite before any skipped slot can
                # be read by a matmul (0 * garbage must stay 0)
                for _i in range(mbufs):
                    mz = mpool.tile([P, g_gather * P], f16, tag="msg")
                    nc.gpsimd.memset(mz[:], 0.0)

            for _rep in range(reps):
                msg_tiles = {}
                oh_tiles = {}
                psum_t = None
                next_chunk = 0
                for t in range(T):
                    w = tile_win[t]
                    tw = tile_tw[t]
                    T_w = T_los[w] + T_his[w]
                    while (next_chunk < len(chunks)
                           and chunks[next_chunk][0] == t):
                        t0, g, hi = chunks[next_chunk]
                        m = mpool.tile([P, g_gather * P], f16, tag="msg")
                        src_view = (table[SPLIT:, :] if hi
                                    else table[:SPLIT, :])
                        if skip_pad:
                            nc.gpsimd.reg_load(
                                cnt_reg,
                                counts_sb[0:1, next_chunk: next_chunk + 1])
                            nreg = cnt_reg
                        else:
                            nreg = g * P
                        nc.gpsimd.dma_gather(
                            out_ap=m[:, : g * P].rearrange(
                                "p (g f) -> p g f", f=P),
                            in_ap=src_view,
                            idxs_ap=idx_sb[:, t0 * 8: (t0 + g) * 8],
                            num_idxs=g * P,
                            num_idxs_reg=nreg,
                            elem_size=F,
                            queue_num=queue_of_chunk[next_chunk],
                        )
                        msg_tiles[next_chunk] = m
                        next_chunk += 1
                    if t % g_oh == 0:
                        g = min(g_oh, T - t)
                        oh = opool.tile([P, g_oh * NW], f16, tag="oh")
                        nc.vector.tensor_tensor(
                            out=oh[:, : g * NW].rearrange(
                                "p (g n) -> p g n", n=NW),
                            in0=dstrel_sb[:, t: t + g].to_broadcast(
                                [P, g, NW]),
                            in1=iota_sb[:, : g * NW].rearrange(
                                "p (g n) -> p g n", n=NW),
                            op=mybir.AluOpType.is_equal,
                        )
                        oh_tiles[t // g_oh] = oh
                    if tw == 0:
                        psum_t = ppool.tile([P, NW], mybir.dt.float32,
                                            tag="ps")
                    ci, jm = chunk_of_tile[t]
                    m = msg_tiles[ci]
                    oh = oh_tiles[t // g_oh]
                    jo = t % g_oh
                    nc.tensor.matmul(
                        out=psum_t[:],
                        lhsT=m[:, jm * P: (jm + 1) * P],
                        rhs=oh[:, jo * NW: (jo + 1) * NW],
                        start=(tw == 0),
                        stop=(tw == T_w - 1),
                    )
                    if tw == T_w - 1:
                        sl = outbuf[:, w * NW: (w + 1) * NW]
                        nc.vector.tensor_copy(out=sl[0:64, :],
                                              in_=psum_t[0:64, :])
                        nc.scalar.activation(
                            out=sl[64:128, :], in_=psum_t[64:128, :],
                            func=mybir.ActivationFunctionType.Exp)
                nc.sync.dma_start(out=out[:], in_=outbuf[:])

    nc.compile()
    _split_multi_waits(nc)
    return nc


def _host_prep(x_sum, x_prod, edge_index):
    n = x_sum.shape[0]
    src = np.ascontiguousarray(edge_index[0]).astype(np.int64)
    dst = np.ascontiguousarray(edge_index[1]).astype(np.int64)
    n_windows_total = -(-n // NW)
    R = n + 2
    hi_pad = R - 1 - SPLIT

    table = np.empty((R, F), np.float16)
    table[1: n + 1, :64] = x_sum.astype(np.float16)
    table[1: n + 1, 64:] = np.log(
        x_prod.astype(np.float64) + LN_BIAS).astype(np.float16)
    table[0, :] = 0.0
    table[n + 1, :] = 0.0

    row = src + 1
    is_hi = row >= SPLIT
    win_all = dst // NW
    order = np.lexsort((dst, is_hi, win_all))
    dst_s = dst[order]
    row_s = row[order]
    hi_s = is_hi[order]
    win = win_all[order]

    counts_all = np.bincount(win, minlength=n_windows_total)
    lo_counts = np.bincount(win[~hi_s], minlength=n_windows_total)
    hi_counts = np.bincount(win[hi_s], minlength=n_windows_total)
    starts = np.zeros(n_windows_total + 1, np.int64)
    np.cumsum(counts_all, out=starts[1:])

    # LPT-pack windows onto devices to balance edge counts (max 25/device),
    # then order each device's windows by descending count so per-local-rank
    # maxima across devices stay tight.
    w_per_dev = -(-n_windows_total // N_DEVICES)
    order_w = np.argsort(-counts_all, kind="stable")
    loads = [0] * N_DEVICES
    slots = [[] for _ in range(N_DEVICES)]
    for w in order_w:
        cands = [d for d in range(N_DEVICES) if len(slots[d]) < w_per_dev]
        d = min(cands, key=lambda i: loads[i])
        slots[d].append(int(w))
        loads[d] += int(counts_all[w])
    # within a device, windows already appended in global descending order
    win_map = slots  # win_map[d][i] = global window id

    T_los, T_his = [], []
    for i in range(w_per_dev):
        lo_m = max((lo_counts[slots[d][i]] for d in range(N_DEVICES)
                    if i < len(slots[d])), default=0)
        hi_m = max((hi_counts[slots[d][i]] for d in range(N_DEVICES)
                    if i < len(slots[d])), default=0)
        T_los.append(max(1, -(-int(lo_m) // P)))
        T_his.append(-(-int(hi_m) // P))
    T_los = tuple(T_los)
    T_his = tuple(T_his)
    T = sum(T_los) + sum(T_his)
    tile_base = np.zeros(w_per_dev + 1, np.int64)
    np.cumsum(np.asarray(T_los) + np.asarray(T_his), out=tile_base[1:])

    chunks = make_chunks(T_los, T_his)

    idx_devs, dstrel_devs, counts_devs = [], [], []
    for d in range(N_DEVICES):
        idx_flat = np.full(T * P, -1, np.int16)
        rel_flat = np.full(T * P, -1.0, np.float16)
        for i in range(w_per_dev):
            if i >= len(slots[d]):
                continue
            w = slots[d][i]
            base = tile_base[i] * P
            T_lo_w = T_los[i]
            a, b = starts[w], starts[w + 1]
            rows_w = row_s[a:b]
            dst_w = dst_s[a:b]
            hi_w = hi_s[a:b]
            nlo = int((~hi_w).sum())
            idx_flat[base: base + nlo] = rows_w[:nlo]
            rel_flat[base: base + nlo] = (dst_w[:nlo] - w * NW).astype(
                np.float16)
            nhi = len(rows_w) - nlo
            hb = base + T_lo_w * P
            idx_flat[hb: hb + nhi] = rows_w[nlo:] - SPLIT
            rel_flat[hb: hb + nhi] = (dst_w[nlo:] - w * NW).astype(
                np.float16)
        cnts = np.zeros(len(chunks), np.int32)
        for ci, (t0, g, hi) in enumerate(chunks):
            sl = idx_flat[t0 * P: (t0 + g) * P]
            v = int((sl >= 0).sum())
            if v == 0:
                sl[0] = hi_pad if hi else 0
                v = 1
            cnts[ci] = v
        counts_devs.append(np.ascontiguousarray(cnts.reshape(1, -1)))
        wrapped = idx_flat.reshape(-1, 16).T
        idx_devs.append(np.ascontiguousarray(np.tile(wrapped, (8, 1))))
        dstrel_devs.append(np.ascontiguousarray(rel_flat.reshape(T, P).T))

    meta = dict(R=R, T=T, n_windows=w_per_dev, T_los=T_los, T_his=T_his,
                n=n, win_map=win_map)
    return table, idx_devs, dstrel_devs, counts_devs, meta


def _make_iota(g_oh=G_OH):
    return np.tile(np.arange(NW, dtype=np.float16), (P, g_oh))


def prep_in_maps(inputs):
    """Host prep for the bench harness: returns (in_maps, build_key, meta)."""
    x_sum = np.ascontiguousarray(np.asarray(inputs["x_sum"], np.float32))
    x_prod = np.ascontiguousarray(np.asarray(inputs["x_prod"], np.float32))
    table, idx_devs, dstrel_devs, counts_devs, meta = _host_prep(
        x_sum, x_prod, inputs["edge_index"])
    iota = _make_iota()
    in_maps = [{"table": table, "idx16": idx_devs[d],
                "dstrel": dstrel_devs[d], "iota": iota,
                "counts": counts_devs[d]}
               for d in range(N_DEVICES)]
    key = (meta["R"], meta["T"], meta["n_windows"], meta["T_los"],
           meta["T_his"])
    return in_maps, key, meta


class _Runner:
    """Execute the Bass module on the 8 axon-tunneled cores via PJRT."""

    def __init__(self, nc, n_cores=N_DEVICES):
        import jax
        from concourse.bass2jax import install_neuronx_cc_hook
        install_neuronx_cc_hook()
        self.jax = jax
        self.nc = nc
        self.n_cores = n_cores
        self.partition_name = (
            nc.partition_id_tensor.name if nc.partition_id_tensor else None)
        in_names, out_names, out_avals, zero_outs = [], [], [], []
        for alloc in nc.m.functions[0].allocations:
            if not isinstance(alloc, mybir.MemoryLocationSet):
                continue
            name = alloc.memorylocations[0].name
            if alloc.kind == "ExternalInput":
                if name == self.partition_name:
                    continue
                in_names.append(name)
            elif alloc.kind == "ExternalOutput":
                out_names.append(name)
                shape = tuple(alloc.tensor_shape)
                dtype = mybir.dt.np(alloc.dtype)
                out_avals.append(jax.core.ShapedArray(shape, dtype))
                zero_outs.append(np.zeros(shape, dtype))
        self.in_names = in_names
        self.out_names = out_names
        self.out_avals = out_avals
        self.zero_outs = zero_outs
        self._jit = None
        self._mesh = None

    def _body(self, *args):
        from concourse.bass2jax import _bass_exec_p, partition_id_tensor
        all_names = self.in_names + self.out_names
        operands = list(args)
        if self.partition_name is not None:
            operands.append(partition_id_tensor())
            all_names = all_names + [self.partition_name]
        outs = _bass_exec_p.bind(
            *operands,
            out_avals=tuple(self.out_avals),
            in_names=tuple(all_names),
            out_names=tuple(self.out_names),
            lowering_input_output_aliases=(),
            sim_require_finite=False,
            sim_require_nnan=False,
            nc=self.nc,
        )
        return tuple(outs)

    def _ensure_jit(self):
        jax = self.jax
        from jax.sharding import Mesh, PartitionSpec
        from jax.experimental.shard_map import shard_map
        if self._jit is None:
            devices = jax.devices()[: self.n_cores]
            self._mesh = Mesh(np.asarray(devices), ("core",))
            n_args = len(self.in_names) + len(self.out_names)
            self._jit = jax.jit(
                shard_map(self._body, mesh=self._mesh,
                          in_specs=(PartitionSpec("core"),) * n_args,
                          out_specs=(PartitionSpec("core"),)
                          * len(self.out_names),
                          check_rep=False),
                keep_unused=True,
            )

    def _concat(self, in_maps):
        concat = [
            np.concatenate([np.asarray(m[name]) for m in in_maps], axis=0)
            for name in self.in_names
        ]
        concat += [np.concatenate([z] * self.n_cores, axis=0)
                   for z in self.zero_outs]
        return concat

    def put(self, in_maps):
        """Upload inputs once; returns device-resident args for run_dev."""
        jax = self.jax
        self._ensure_jit()
        from jax.sharding import NamedSharding, PartitionSpec
        sh = NamedSharding(self._mesh, PartitionSpec("core"))
        return [jax.device_put(c, sh) for c in self._concat(in_maps)]

    def run_dev(self, dev_args):
        jax = self.jax
        return jax.block_until_ready(self._jit(*dev_args))

    def run(self, in_maps):
        jax = self.jax
        self._ensure_jit()
        outs = jax.block_until_ready(self._jit(*self._concat(in_maps)))
        results = []
        for c in range(self.n_cores):
            results.append({
                name: np.asarray(outs[i]).reshape(
                    self.n_cores, *self.out_avals[i].shape)[c]
                for i, name in enumerate(self.out_names)})
        return results


_CACHE = {}


def kernel(x_sum, x_prod, edge_index):
    x_sum = np.ascontiguousarray(np.asarray(x_sum, dtype=np.float32))
    x_prod = np.ascontiguousarray(np.asarray(x_prod, dtype=np.float32))
    table, idx_devs, dstrel_devs, counts_devs, meta = _host_prep(
        x_sum, x_prod, edge_index)
    iota = _make_iota()

    key = (meta["R"], meta["T"], meta["n_windows"], meta["T_los"],
           meta["T_his"])
    if key not in _CACHE:
        nc = _build_kernel(*key)
        _CACHE[key] = _Runner(nc)
    runner = _CACHE[key]

    in_maps = [{"table": table, "idx16": idx_devs[d],
                "dstrel": dstrel_devs[d], "iota": iota,
                "counts": counts_devs[d]}
               for d in range(N_DEVICES)]
    n = meta["n"]
    w_per_dev = meta["n_windows"]
    win_map = meta["win_map"]
    for _attempt in range(3):
        results = runner.run(in_maps)
        full = np.empty((P, n), np.float32)
        ok = True
        for d in range(N_DEVICES):
            o = results[d]["out"]
            for i, w in enumerate(win_map[d]):
                c0 = w * NW
                c1 = min(c0 + NW, n)
                full[:, c0:c1] = o[:, i * NW: i * NW + (c1 - c0)]
        if np.isfinite(full).all():
            break
    out_sum = np.ascontiguousarray(full[:64].T)
    out_prod = np.ascontiguousarray(full[64:].T)
    return out_sum, out_prod


# revision 8
# speedup vs baseline: 1.5535x; 1.5535x over previous
"""GNN message passing (segment-sum + segment-product) on 8 TRN2 NeuronCores.

Strategy (node sharding, no collectives):
- dst nodes are grouped into 256-node windows; windows are LPT-packed onto
  the 8 devices to balance edge counts (the host scatters outputs back by a
  window map). Each device only computes rows for windows it owns.
- The node-feature table is replicated: fp16 rows [64 x_sum feats |
  64 ln(x_prod + 1e-38) feats] (ln precomputed on host, so the device never
  runs Ln; the gather is descriptor-rate-bound, so fp16's smaller rows are
  free and halve vector-engine work downstream).
- Edges are fetched with GPSIMD dma_gather (256B rows), chunks alternating
  across 2 SWDGE queues (the per-queue descriptor processor is the serial
  resource; two queues run in parallel). dma_gather indices are int16, so
  the table is addressed through two views (rows < 32768 and the rest);
  each window's edges are ordered low-rows-first.
- Padding slots are skipped: their idx is -1 (trailing negatives are not
  processed by the gather) and the true per-chunk count is loaded into a
  Pool register for num_idxs_reg, so each device gathers only real edges.
  Padding dstrel is -1, which zeroes the one-hot column, so whatever is in
  the skipped SBUF slots contributes nothing (the message arena is memset
  once at start so it is always finite).
- Segment-sum is a one-hot matmul: onehot[e, n] = (dstrel[e] == n) built by
  the vector engine in fp16, then PE computes msg.T @ onehot accumulated in
  PSUM per window. The product path accumulates ln-sums; Act applies Exp on
  evacuation.
- Device output is feature-major [128, windows*256]; rows 0:64 sums, rows
  64:128 products. The host scatters window columns back and transposes.
"""

import time

import numpy as np

import concourse.bacc as bacc
import concourse.mybir as mybir
import concourse.tile as tile

F = 128          # table row width: 64 sum feats | 64 ln(prod) feats
P = 128
NW = 256         # dst nodes per window
SPLIT = 32768    # int16 index limit for dma_gather
LN_BIAS = 1e-38
N_DEVICES = 8
G_GATHER = 7     # tiles per dma_gather
G_OH = 7         # tiles per one-hot batch
NQ = 2           # SWDGE queues

_MAX_WAITS = 1   # this walrus build allows one sync wait per instruction


def _split_multi_waits(nc):
    """Split instructions carrying more sem waits than walrus accepts."""
    for fn in nc.m.functions:
        for bb in fn.blocks:
            insts = list(bb.instructions)
            new_insts = []
            changed = False
            for inst in insts:
                si = inst.sync_info
                if si is not None and len(si.on_wait) > _MAX_WAITS:
                    waits = list(si.on_wait)
                    k = 0
                    while len(waits) > _MAX_WAITS:
                        chunk, waits = waits[:_MAX_WAITS], waits[_MAX_WAITS:]
                        helper = mybir.InstDrain(
                            name=f"{inst.name}_ws{k}", engine=inst.engine)
                        helper.sync_info = mybir.SyncInfo(
                            on_wait=chunk, on_update=[])
                        new_insts.append(helper)
                        k += 1
                    inst.sync_info = mybir.SyncInfo(
                        on_wait=waits, on_update=list(si.on_update))
                    changed = True
                new_insts.append(inst)
            if changed:
                bb.instructions = new_insts


def make_chunks(T_los, T_his, g_gather=G_GATHER):
    """Gather chunk list shared by builder and host prep: (t0, g, hi)."""
    run_list = []
    pos = 0
    for w in range(len(T_los)):
        run_list.append((pos, T_los[w], False))
        run_list.append((pos + T_los[w], T_his[w], True))
        pos += T_los[w] + T_his[w]
    chunks = []
    for run_start, run_len, hi in run_list:
        if run_len == 0:
            continue
        n_chunks = -(-run_len // g_gather)
        base_sz, extra = divmod(run_len, n_chunks)
        t0 = run_start
        for j in range(n_chunks):
            g = base_sz + (1 if j < extra else 0)
            chunks.append((t0, g, hi))
            t0 += g
    return chunks


def _build_kernel(R, T, n_windows, T_los, T_his, reps=1, nq=NQ,
                  g_gather=G_GATHER, g_oh=G_OH, mbufs=6, obufs=3, pbufs=8,
                  outbufs=2, skip_pad=True):
    T_los = tuple(T_los)
    T_his = tuple(T_his)
    assert T == sum(T_los) + sum(T_his)
    tile_win = []
    tile_tw = []
    for w in range(n_windows):
        for tw in range(T_los[w] + T_his[w]):
            tile_win.append(w)
            tile_tw.append(tw)
    chunks = make_chunks(T_los, T_his, g_gather)

    nc = bacc.Bacc("TRN2", target_bir_lowering=False, debug=False,
                   num_swdge_queues=nq)
    f32 = mybir.dt.float32
    f16 = mybir.dt.float16
    i16 = mybir.dt.int16

    table = nc.dram_tensor("table", [R, F], f16, kind="ExternalInput").ap()
    idx16 = nc.dram_tensor("idx16", [P, T * 8], i16,
                           kind="ExternalInput").ap()
    dstrel = nc.dram_tensor("dstrel", [P, T], f16, kind="ExternalInput").ap()
    iota = nc.dram_tensor("iota", [P, g_oh * NW], f16,
                          kind="ExternalInput").ap()
    counts = nc.dram_tensor("counts", [1, len(chunks)], mybir.dt.int32,
                            kind="ExternalInput").ap()
    out = nc.dram_tensor("out", [P, n_windows * NW], f32,
                         kind="ExternalOutput").ap()

    with tile.TileContext(nc) as tc:
        with (
            tc.tile_pool(name="const", bufs=1) as cpool,
            tc.tile_pool(name="msg", bufs=mbufs) as mpool,
            tc.tile_pool(name="oh", bufs=obufs) as opool,
            tc.tile_pool(name="outb", bufs=outbufs) as outpool,
            tc.tile_pool(name="psum", bufs=pbufs, space="PSUM") as ppool,
        ):
            idx_sb = cpool.tile([P, T * 8], i16, tag="idx")
            counts_sb = cpool.tile([1, len(chunks)], mybir.dt.int32,
                                   tag="counts")
            dstrel_sb = cpool.tile([P, T], f16, tag="dstrel")
            iota_sb = cpool.tile([P, g_oh * NW], f16, tag="iota")
            nc.sync.dma_start(out=idx_sb[:], in_=idx16[:])
            nc.sync.dma_start(out=counts_sb[:], in_=counts[:])
            nc.sync.dma_start(out=dstrel_sb[:], in_=dstrel[:])
            nc.sync.dma_start(out=iota_sb[:], in_=iota[:])
            cnt_reg = nc.gpsimd.alloc_register("gather_cnt")

            chunk_of_tile = {}
            for ci, (t0, g, hi) in enumerate(chunks):
                for j in range(g):
                    chunk_of_tile[t0 + j] = (ci, j)
            queue_of_chunk = []
            qload = [0] * nq
            for (t0, g, hi) in chunks:
                q = min(range(nq), key=lambda i: qload[i])
                queue_of_chunk.append(q)
                qload[q] += g

            if skip_pad:
                # make the message arena finite before any skipped slot can
                # be read by a matmul (0 * garbage must stay 0)
                for _i in range(mbufs):
                    mz = mpool.tile([P, g_gather * P], f16, tag="msg")
                    nc.gpsimd.memset(mz[:], 0.0)

            for _rep in range(reps):
                outbuf = outpool.tile([P, n_windows * NW], f32,
                                      tag="outbuf")
                msg_tiles = {}
                oh_tiles = {}
                psum_t = None
                next_chunk = 0
                for t in range(T):
                    w = tile_win[t]
                    tw = tile_tw[t]
                    T_w = T_los[w] + T_his[w]
                    while (next_chunk < len(chunks)
                           and chunks[next_chunk][0] == t):
                        t0, g, hi = chunks[next_chunk]
                        m = mpool.tile([P, g_gather * P], f16, tag="msg")
                        src_view = (table[SPLIT:, :] if hi
                                    else table[:SPLIT, :])
                        if skip_pad:
                            nc.gpsimd.reg_load(
                                cnt_reg,
                                counts_sb[0:1, next_chunk: next_chunk + 1])
                            nreg = cnt_reg
                        else:
                            nreg = g * P
                        nc.gpsimd.dma_gather(
                            out_ap=m[:, : g * P].rearrange(
                                "p (g f) -> p g f", f=P),
                            in_ap=src_view,
                            idxs_ap=idx_sb[:, t0 * 8: (t0 + g) * 8],
                            num_idxs=g * P,
                            num_idxs_reg=nreg,
                            elem_size=F,
                            queue_num=queue_of_chunk[next_chunk],
                        )
                        msg_tiles[next_chunk] = m
                        next_chunk += 1
                    if t % g_oh == 0:
                        g = min(g_oh, T - t)
                        oh = opool.tile([P, g_oh * NW], f16, tag="oh")
                        nc.vector.tensor_tensor(
                            out=oh[:, : g * NW].rearrange(
                                "p (g n) -> p g n", n=NW),
                            in0=dstrel_sb[:, t: t + g].to_broadcast(
                                [P, g, NW]),
                            in1=iota_sb[:, : g * NW].rearrange(
                                "p (g n) -> p g n", n=NW),
                            op=mybir.AluOpType.is_equal,
                        )
                        oh_tiles[t // g_oh] = oh
                    if tw == 0:
                        psum_t = ppool.tile([P, NW], mybir.dt.float32,
                                            tag="ps")
                    ci, jm = chunk_of_tile[t]
                    m = msg_tiles[ci]
                    oh = oh_tiles[t // g_oh]
                    jo = t % g_oh
                    nc.tensor.matmul(
                        out=psum_t[:],
                        lhsT=m[:, jm * P: (jm + 1) * P],
                        rhs=oh[:, jo * NW: (jo + 1) * NW],
                        start=(tw == 0),
                        stop=(tw == T_w - 1),
                    )
                    if tw == T_w - 1:
                        sl = outbuf[:, w * NW: (w + 1) * NW]
                        nc.vector.tensor_copy(out=sl[0:64, :],
                                              in_=psum_t[0:64, :])
                        nc.scalar.activation(
                            out=sl[64:128, :], in_=psum_t[64:128, :],
                            func=mybir.ActivationFunctionType.Exp)
                nc.sync.dma_start(out=out[:], in_=outbuf[:])

    nc.compile()
    _split_multi_waits(nc)
    return nc


def _host_prep(x_sum, x_prod, edge_index):
    n = x_sum.shape[0]
    src = np.ascontiguousarray(edge_index[0]).astype(np.int64)
    dst = np.ascontiguousarray(edge_index[1]).astype(np.int64)
    n_windows_total = -(-n // NW)
    R = n + 2
    hi_pad = R - 1 - SPLIT

    table = np.empty((R, F), np.float16)
    table[1: n + 1, :64] = x_sum.astype(np.float16)
    table[1: n + 1, 64:] = np.log(
        x_prod.astype(np.float64) + LN_BIAS).astype(np.float16)
    table[0, :] = 0.0
    table[n + 1, :] = 0.0

    row = src + 1
    is_hi = row >= SPLIT
    win_all = dst // NW
    order = np.lexsort((dst, is_hi, win_all))
    dst_s = dst[order]
    row_s = row[order]
    hi_s = is_hi[order]
    win = win_all[order]

    counts_all = np.bincount(win, minlength=n_windows_total)
    lo_counts = np.bincount(win[~hi_s], minlength=n_windows_total)
    hi_counts = np.bincount(win[hi_s], minlength=n_windows_total)
    starts = np.zeros(n_windows_total + 1, np.int64)
    np.cumsum(counts_all, out=starts[1:])

    # LPT-pack windows onto devices to balance edge counts (max 25/device),
    # then order each device's windows by descending count so per-local-rank
    # maxima across devices stay tight.
    w_per_dev = -(-n_windows_total // N_DEVICES)
    order_w = np.argsort(-counts_all, kind="stable")
    loads = [0] * N_DEVICES
    slots = [[] for _ in range(N_DEVICES)]
    for w in order_w:
        cands = [d for d in range(N_DEVICES) if len(slots[d]) < w_per_dev]
        d = min(cands, key=lambda i: loads[i])
        slots[d].append(int(w))
        loads[d] += int(counts_all[w])
    # within a device, windows already appended in global descending order
    win_map = slots  # win_map[d][i] = global window id

    T_los, T_his = [], []
    for i in range(w_per_dev):
        lo_m = max((lo_counts[slots[d][i]] for d in range(N_DEVICES)
                    if i < len(slots[d])), default=0)
        hi_m = max((hi_counts[slots[d][i]] for d in range(N_DEVICES)
                    if i < len(slots[d])), default=0)
        T_los.append(max(1, -(-int(lo_m) // P)))
        T_his.append(-(-int(hi_m) // P))
    T_los = tuple(T_los)
    T_his = tuple(T_his)
    T = sum(T_los) + sum(T_his)
    tile_base = np.zeros(w_per_dev + 1, np.int64)
    np.cumsum(np.asarray(T_los) + np.asarray(T_his), out=tile_base[1:])

    chunks = make_chunks(T_los, T_his)

    idx_devs, dstrel_devs, counts_devs = [], [], []
    for d in range(N_DEVICES):
        idx_flat = np.full(T * P, -1, np.int16)
        rel_flat = np.full(T * P, -1.0, np.float16)
        for i in range(w_per_dev):
            if i >= len(slots[d]):
                continue
            w = slots[d][i]
            base = tile_base[i] * P
            T_lo_w = T_los[i]
            a, b = starts[w], starts[w + 1]
            rows_w = row_s[a:b]
            dst_w = dst_s[a:b]
            hi_w = hi_s[a:b]
            nlo = int((~hi_w).sum())
            idx_flat[base: base + nlo] = rows_w[:nlo]
            rel_flat[base: base + nlo] = (dst_w[:nlo] - w * NW).astype(
                np.float16)
            nhi = len(rows_w) - nlo
            hb = base + T_lo_w * P
            idx_flat[hb: hb + nhi] = rows_w[nlo:] - SPLIT
            rel_flat[hb: hb + nhi] = (dst_w[nlo:] - w * NW).astype(
                np.float16)
        cnts = np.zeros(len(chunks), np.int32)
        for ci, (t0, g, hi) in enumerate(chunks):
            sl = idx_flat[t0 * P: (t0 + g) * P]
            v = int((sl >= 0).sum())
            if v == 0:
                sl[0] = hi_pad if hi else 0
                v = 1
            cnts[ci] = v
        counts_devs.append(np.ascontiguousarray(cnts.reshape(1, -1)))
        wrapped = idx_flat.reshape(-1, 16).T
        idx_devs.append(np.ascontiguousarray(np.tile(wrapped, (8, 1))))
        dstrel_devs.append(np.ascontiguousarray(rel_flat.reshape(T, P).T))

    meta = dict(R=R, T=T, n_windows=w_per_dev, T_los=T_los, T_his=T_his,
                n=n, win_map=win_map)
    return table, idx_devs, dstrel_devs, counts_devs, meta


def _make_iota(g_oh=G_OH):
    return np.tile(np.arange(NW, dtype=np.float16), (P, g_oh))


def prep_in_maps(inputs):
    """Host prep for the bench harness: returns (in_maps, build_key, meta)."""
    x_sum = np.ascontiguousarray(np.asarray(inputs["x_sum"], np.float32))
    x_prod = np.ascontiguousarray(np.asarray(inputs["x_prod"], np.float32))
    table, idx_devs, dstrel_devs, counts_devs, meta = _host_prep(
        x_sum, x_prod, inputs["edge_index"])
    iota = _make_iota()
    in_maps = [{"table": table, "idx16": idx_devs[d],
                "dstrel": dstrel_devs[d], "iota": iota,
                "counts": counts_devs[d]}
               for d in range(N_DEVICES)]
    key = (meta["R"], meta["T"], meta["n_windows"], meta["T_los"],
           meta["T_his"])
    return in_maps, key, meta


class _Runner:
    """Execute the Bass module on the 8 axon-tunneled cores via PJRT."""

    def __init__(self, nc, n_cores=N_DEVICES):
        import jax
        from concourse.bass2jax import install_neuronx_cc_hook
        install_neuronx_cc_hook()
        self.jax = jax
        self.nc = nc
        self.n_cores = n_cores
        self.partition_name = (
            nc.partition_id_tensor.name if nc.partition_id_tensor else None)
        in_names, out_names, out_avals, zero_outs = [], [], [], []
        for alloc in nc.m.functions[0].allocations:
            if not isinstance(alloc, mybir.MemoryLocationSet):
                continue
            name = alloc.memorylocations[0].name
            if alloc.kind == "ExternalInput":
                if name == self.partition_name:
                    continue
                in_names.append(name)
            elif alloc.kind == "ExternalOutput":
                out_names.append(name)
                shape = tuple(alloc.tensor_shape)
                dtype = mybir.dt.np(alloc.dtype)
                out_avals.append(jax.core.ShapedArray(shape, dtype))
                zero_outs.append(np.zeros(shape, dtype))
        self.in_names = in_names
        self.out_names = out_names
        self.out_avals = out_avals
        self.zero_outs = zero_outs
        self._jit = None
        self._mesh = None

    def _body(self, *args):
        from concourse.bass2jax import _bass_exec_p, partition_id_tensor
        all_names = self.in_names + self.out_names
        operands = list(args)
        if self.partition_name is not None:
            operands.append(partition_id_tensor())
            all_names = all_names + [self.partition_name]
        outs = _bass_exec_p.bind(
            *operands,
            out_avals=tuple(self.out_avals),
            in_names=tuple(all_names),
            out_names=tuple(self.out_names),
            lowering_input_output_aliases=(),
            sim_require_finite=False,
            sim_require_nnan=False,
            nc=self.nc,
        )
        return tuple(outs)

    def _ensure_jit(self):
        jax = self.jax
        from jax.sharding import Mesh, PartitionSpec
        from jax.experimental.shard_map import shard_map
        if self._jit is None:
            devices = jax.devices()[: self.n_cores]
            self._mesh = Mesh(np.asarray(devices), ("core",))
            n_args = len(self.in_names) + len(self.out_names)
            self._jit = jax.jit(
                shard_map(self._body, mesh=self._mesh,
                          in_specs=(PartitionSpec("core"),) * n_args,
                          out_specs=(PartitionSpec("core"),)
                          * len(self.out_names),
                          check_rep=False),
                keep_unused=True,
            )

    def _concat(self, in_maps):
        concat = [
            np.concatenate([np.asarray(m[name]) for m in in_maps], axis=0)
            for name in self.in_names
        ]
        concat += [np.concatenate([z] * self.n_cores, axis=0)
                   for z in self.zero_outs]
        return concat

    def put(self, in_maps):
        """Upload inputs once; returns device-resident args for run_dev."""
        jax = self.jax
        self._ensure_jit()
        from jax.sharding import NamedSharding, PartitionSpec
        sh = NamedSharding(self._mesh, PartitionSpec("core"))
        return [jax.device_put(c, sh) for c in self._concat(in_maps)]

    def run_dev(self, dev_args):
        jax = self.jax
        return jax.block_until_ready(self._jit(*dev_args))

    def run(self, in_maps):
        jax = self.jax
        self._ensure_jit()
        outs = jax.block_until_ready(self._jit(*self._concat(in_maps)))
        results = []
        for c in range(self.n_cores):
            results.append({
                name: np.asarray(outs[i]).reshape(
                    self.n_cores, *self.out_avals[i].shape)[c]
                for i, name in enumerate(self.out_names)})
        return results


_CACHE = {}


def kernel(x_sum, x_prod, edge_index):
    x_sum = np.ascontiguousarray(np.asarray(x_sum, dtype=np.float32))
    x_prod = np.ascontiguousarray(np.asarray(x_prod, dtype=np.float32))
    table, idx_devs, dstrel_devs, counts_devs, meta = _host_prep(
        x_sum, x_prod, edge_index)
    iota = _make_iota()

    key = (meta["R"], meta["T"], meta["n_windows"], meta["T_los"],
           meta["T_his"])
    if key not in _CACHE:
        nc = _build_kernel(*key)
        _CACHE[key] = _Runner(nc)
    runner = _CACHE[key]

    in_maps = [{"table": table, "idx16": idx_devs[d],
                "dstrel": dstrel_devs[d], "iota": iota,
                "counts": counts_devs[d]}
               for d in range(N_DEVICES)]
    n = meta["n"]
    w_per_dev = meta["n_windows"]
    win_map = meta["win_map"]
    for _attempt in range(3):
        results = runner.run(in_maps)
        full = np.empty((P, n), np.float32)
        ok = True
        for d in range(N_DEVICES):
            o = results[d]["out"]
            for i, w in enumerate(win_map[d]):
                c0 = w * NW
                c1 = min(c0 + NW, n)
                full[:, c0:c1] = o[:, i * NW: i * NW + (c1 - c0)]
        if np.isfinite(full).all():
            break
    out_sum = np.ascontiguousarray(full[:64].T)
    out_prod = np.ascontiguousarray(full[64:].T)
    return out_sum, out_prod
